# revision 64
# baseline (speedup 1.0000x reference)
"""Distributed GQA causal self-attention (RoPE + RMSNorm QK) for 8 TRN2 cores.

Sharding: DP=2 over batch x TP=4 over KV-head groups.
Core c = 4*b + s handles batch b, kv-group s (1 kv head, 4 q heads).
Per-batch ReduceScatter (replica groups [[0..3],[4..7]]) after the output
projection; the host concatenates the 8 scattered shards.

Layout: features on partitions, tokens on the free axis; the only on-device
transpose is tiny (vT -> v):
  qT = WqT.T @ xT          (256, T)   scoresT = knT.T @ qnT   (kt, qt)
  softmax over kt via exp + matmul with ones-augmented v (row 64 = sums)
  yT = v_aug.T @ expT      (65, qt)   outT = WoT.T @ ynT      (1024, T)
RMS-normed q,k bound scores to |s| <= 8, so exp needs no max subtraction.

All matmul operands are bf16 (incl. the sum-of-squares pass; fp32 rhs costs
4 cycles/row on the PE). Causal structure is exploited at 128-column
granularity inside the diagonal 512-blocks: scores/exp/y all skip the
always-masked query columns. RMS-norm: PE sum-of-squares -> DVE approx
reciprocal -> one batched ACT Sqrt per chunk (sqrt(1/m)), so the scalar
engine only ever alternates Sqrt/Exp once per chunk.
"""

import numpy as np
from contextlib import ExitStack

B, T, C = 2, 2048, 1024
NH, NKV, HD, HALF = 16, 4, 64, 32
G = NH // NKV          # 4 q heads per kv head
TP, DP = 4, 2
KC = C // 128          # 8 contraction tiles
NT = T // 128          # 16 token tiles
NQ = T // 512          # 4 query chunks
SCALE = 1.0 / np.sqrt(HD)
VS = 72                # column stride of packed v blocks (65 used)

_CACHE = {}
SIM_MODE = False


def _build():
    import concourse.bass as bass
    import concourse.bacc as bacc
    import concourse.mybir as mybir
    import concourse.tile as tile

    f32 = mybir.dt.float32
    bf16 = mybir.dt.bfloat16
    AF = mybir.ActivationFunctionType

    nc = bacc.Bacc("TRN2", target_bir_lowering=False, debug=False,
                   num_devices=8)

    xT = nc.dram_tensor("xT", [C, T], bf16, kind="ExternalInput").ap()
    cosT = nc.dram_tensor("cosT", [128, T], bf16, kind="ExternalInput").ap()
    sinT = nc.dram_tensor("sinT", [128, T], bf16, kind="ExternalInput").ap()
    wqT = nc.dram_tensor("wqT", [C, G * HD], bf16, kind="ExternalInput").ap()
    wkvT = nc.dram_tensor("wkvT", [C, 2 * HD], bf16,
                          kind="ExternalInput").ap()
    woT = nc.dram_tensor("woT", [G * HD, C], bf16, kind="ExternalInput").ap()
    masks = nc.dram_tensor("masks", [128, 128], bf16,
                           kind="ExternalInput").ap()
    eye64 = nc.dram_tensor("eye64", [64, 64], bf16, kind="ExternalInput").ap()
    outT = nc.dram_tensor("outT", [C // 4, T], f32, kind="ExternalOutput").ap()

    with tile.TileContext(nc) as tc, ExitStack() as es:
        const = es.enter_context(tc.tile_pool(name="const", bufs=1))
        actp = es.enter_context(tc.tile_pool(name="acts", bufs=1))
        dram = es.enter_context(tc.tile_pool(name="dram", bufs=1, space="DRAM"))

        # ---- persistent weights / tables (merged DMAs) ----
        # DMA order matters: chunk-0 x and Wq go first so the first
        # projection matmuls start as early as possible; Wo last.
        wq_all = const.tile([128, KC * 256], bf16, name="wq", tag="wq")
        wkv_all = const.tile([128, KC * 128], bf16, name="wkv", tag="wkv")
        wo_all = const.tile([128, 2 * C], bf16, name="wo", tag="wo")
        cos_sb = const.tile([128, T], bf16, name="cos", tag="cos")
        sin_sb = const.tile([128, T], bf16, name="sin", tag="sin")
        mask_sb = const.tile([128, 128], bf16, name="mask", tag="mask")
        eye_sb = const.tile([128, 64], bf16, name="eye", tag="eye")

        def weight_dmas_front():
            nc.sync.dma_start(
                wq_all[:].rearrange("p (k m) -> p k m", m=256),
                wqT.rearrange("(k p) m -> p k m", p=128))
            nc.sync.dma_start(
                wkv_all[:].rearrange("p (k m) -> p k m", m=128),
                wkvT.rearrange("(k p) m -> p k m", p=128))
            nc.sync.dma_start(cos_sb[:], cosT)
            nc.sync.dma_start(sin_sb[:], sinT)
            nc.sync.dma_start(eye_sb[64:128, :], eye64)
            nc.sync.dma_start(mask_sb[:], masks)

        def weight_dmas_back():
            nc.sync.dma_start(
                wo_all[:].rearrange("p (k m) -> p k m", m=C),
                woT.rearrange("(k p) m -> p k m", p=128))

        # head-pair selectors for the sum-of-squares matmuls (bf16!)
        onesel = const.tile([128, 2], bf16, name="onesel", tag="onesel")
        nc.any.memset(onesel[:], 0.0)
        nc.any.memset(onesel[0:HALF, 0:1], 1.0)
        nc.any.memset(onesel[HD:HD + HALF, 0:1], 1.0)
        nc.any.memset(onesel[HALF:HD, 1:2], 1.0)
        nc.any.memset(onesel[HD + HALF:128, 1:2], 1.0)
        ones64 = const.tile([64, 1], bf16, name="ones64", tag="ones64")
        nc.any.memset(ones64[:], 1.0)

        # ---- persistent activations ----
        q_raw = [actp.tile([128, T], bf16, name=f"qraw{m}", tag=f"qraw{m}")
                 for m in range(2)]
        qnT = [actp.tile([HD, T], bf16, name=f"qn{h}", tag=f"qn{h}")
               for h in range(G)]
        knT = actp.tile([HD, T], bf16, name="kn", tag="kn")
        v_all = actp.tile([128, NT * VS], bf16, name="vall", tag="vall")
        nc.any.memset(v_all[:, HD::VS], 1.0)   # ones column of each v block
        ynT = [actp.tile([128, T], bf16, name=f"yn{m}", tag=f"yn{m}")
               for m in range(2)]
        # rms staging: engine APs need partition bases in {0,32,64}, so
        # heads 0-2 sit at rows {0,32,64} of tile A and head 3 / k at rows
        # {0,32} of tile B; the gaps are memset once so the batched Sqrts
        # read defined values.
        minvA = actp.tile([65, 512], f32, name="minvA", tag="minvA")
        nc.any.memset(minvA[:], 1.0)
        minvB = actp.tile([33, 512], f32, name="minvB", tag="minvB")
        nc.any.memset(minvB[:], 1.0)
        rrowA = actp.tile([65, 512], bf16, name="rrowA", tag="rrowA")
        rrowB = actp.tile([33, 512], bf16, name="rrowB", tag="rrowB")

        with tc.tile_pool(name="xTp", bufs=2) as xpool, \
             tc.tile_pool(name="kvp", bufs=3) as kvpool, \
             tc.tile_pool(name="rp", bufs=3) as rp, \
             tc.tile_pool(name="nrm", bufs=3) as nrm, \
             tc.tile_pool(name="expp", bufs=6) as expp, \
             tc.tile_pool(name="smx", bufs=3) as smx, \
             tc.tile_pool(name="ps1", bufs=1, space="PSUM") as ps1, \
             tc.tile_pool(name="pss", bufs=3, space="PSUM") as pss, \
             tc.tile_pool(name="psy", bufs=1, space="PSUM") as psy, \
             tc.tile_pool(name="pso", bufs=2, space="PSUM") as pso:

            def rope6(src, W, c0, c1, rT, tP, t0=None, t1=None):
                """rT[0:W]=x1*cos+x2*sin ; rT[W:2W]=x2*cos-x1*sin.

                cos_sb/sin_sb are stacked x4 so x2 (at partition base W)
                multiplies against same-base cos/sin rows — no align copy.
                Product outputs land at whatever base the add/sub needs
                (out base may differ from input base; input bases must
                match each other)."""
                if t0 is None:
                    t0, t1 = c0, c1
                x1 = src[0:W, c0:c1]
                x2 = src[W:2 * W, c0:c1]
                cs, sn = cos_sb[0:W, t0:t1], sin_sb[0:W, t0:t1]
                cs2, sn2 = cos_sb[W:2 * W, t0:t1], sin_sb[W:2 * W, t0:t1]
                nc.vector.tensor_mul(rT[0:W, :], x1, cs)
                nc.vector.tensor_mul(tP[0:W, :], x2, sn2)
                nc.vector.tensor_add(rT[0:W, :], rT[0:W, :], tP[0:W, :])
                nc.vector.tensor_mul(rT[W:2 * W, :], x2, cs2)
                nc.vector.tensor_mul(tP[W:2 * W, :], x1, sn)
                nc.vector.tensor_sub(rT[W:2 * W, :], rT[W:2 * W, :],
                                     tP[W:2 * W, :])

            def proj_part(n, xt):
                c0, c1 = n * 512, (n + 1) * 512
                # q projection (2 m-tiles) + packed kv projection
                for m in range(2):
                    qp = ps1.tile([128, 512], f32, name="pq", tag="pq")
                    for k in range(KC):
                        nc.tensor.matmul(
                            qp[:],
                            lhsT=wq_all[:, k * 256 + m * 128:
                                        k * 256 + (m + 1) * 128],
                            rhs=xt[:, k * 512:(k + 1) * 512],
                            start=(k == 0), stop=(k == KC - 1))
                    nc.scalar.copy(q_raw[m][:, c0:c1], qp[:])
                kvp = ps1.tile([128, 512], f32, name="pkv", tag="pq")
                for k in range(KC):
                    nc.tensor.matmul(
                        kvp[:], lhsT=wkv_all[:, k * 128:(k + 1) * 128],
                        rhs=xt[:, k * 512:(k + 1) * 512],
                        start=(k == 0), stop=(k == KC - 1))
                kvr = kvpool.tile([128, 512], bf16, name="kvr", tag="kvr")
                nc.scalar.copy(kvr[:], kvp[:])

                # v transpose: 4 token tiles -> one psum bank -> v_all blocks
                tp = ps1.tile([128, 4 * HD], bf16, name="tp", tag="ss")
                for it in range(4):
                    nc.tensor.matmul(
                        tp[:, it * HD:(it + 1) * HD],
                        lhsT=kvr[HD:128, it * 128:(it + 1) * 128],
                        rhs=eye_sb[64:128, :], is_transpose=True,
                        skip_group_check=(it > 0))
                nc.vector.tensor_copy(
                    v_all[:].rearrange("p (i s) -> p i s", s=VS)
                    [:, 4 * n:4 * n + 4, 0:HD],
                    tp[:].rearrange("p (i c) -> p i c", c=HD))
                return kvr

            def rope_core(n, kvr):
                c0, c1 = n * 512, (n + 1) * 512
                # rope (2 q pairs + k) + sum-of-squares -> rms factors
                sq = []
                for m in range(2):
                    rT = rp.tile([128, 512], bf16, name=f"rq{m}",
                                 tag=f"rq{m}")
                    tP = rp.tile([128, 512], bf16, name=f"tq{m}",
                                 tag=f"tq{m}")
                    rope6(q_raw[m], HD, c0, c1, rT, tP)
                    s = rp.tile([128, 512], bf16, name=f"sq{m}",
                                tag=f"sq{m}")
                    nc.vector.tensor_mul(s[:], rT[:], rT[:])
                    sq.append((rT, s))
                rTk = rp.tile([64, 512], bf16, name="rk", tag="rk")
                tPk = rp.tile([64, 512], bf16, name="tk", tag="tk")
                rope6(kvr, HALF, 0, 512, rTk, tPk, t0=c0, t1=c1)
                sk = rp.tile([64, 512], bf16, name="sk", tag="sk")
                nc.vector.tensor_mul(sk[:], rTk[:], rTk[:])

                # ss matmuls (M=1 each, psum rows at {0,32,64});
                # DVE approx recip -> minv; two batched Sqrts
                ssqA = ps1.tile([65, 512], f32, name="ssA", tag="ss")
                ssqB = ps1.tile([65, 512], f32, name="ssB", tag="ss")
                # (tile, row) per head 0..3, then k
                slots = [(ssqA, minvA, 0), (ssqA, minvA, 32),
                         (ssqA, minvA, 64), (ssqB, minvB, 0)]
                for h in range(4):
                    m, j = h // 2, h % 2
                    ssq, mv, b = slots[h]
                    nc.tensor.matmul(ssq[b:b + 1, :],
                                     lhsT=onesel[:, j:j + 1],
                                     rhs=sq[m][1][:],
                                     start=True, stop=True,
                                     skip_group_check=(ssq is ssqA
                                                       and b > 0))
                    nc.vector.reciprocal(
                        mv[b:b + 1, :], ssq[b:b + 1, :])
                nc.tensor.matmul(ssqB[32:33, :], lhsT=ones64[:], rhs=sk[:],
                                 start=True, stop=True,
                                 skip_group_check=True)
                nc.vector.reciprocal(minvB[32:33, :],
                                                 ssqB[32:33, :])
                # rrow = sqrt(64 / ss) = rsqrt(mean(x^2))
                nc.scalar.activation(rrowA[:], minvA[:], AF.Sqrt,
                                     scale=float(HD))
                nc.scalar.activation(rrowB[:], minvB[:], AF.Sqrt,
                                     scale=float(HD))
                return sq, rTk

            def norm_finish(n, rts):
                c0, c1 = n * 512, (n + 1) * 512
                sq, rTk = rts
                # normalize: qnT[h] and knT
                # partition_broadcast inputs MUST sit at partition 0 of a
                # tile (offset inputs silently mis-lower in walrus), so rows
                # at base 32/64 are staged through base-0 copies first.
                rsrc = [(rrowA, 0), (rrowA, 32), (rrowA, 64), (rrowB, 0)]
                for m in range(2):
                    rT = sq[m][0]
                    for j in range(2):
                        h = 2 * m + j
                        rr, rb = rsrc[h]
                        if rb > 0:
                            rj = nrm.tile([1, 512], bf16, name=f"rj{h}",
                                          tag=f"rj{h}")
                            nc.vector.tensor_copy(rj[:], rr[rb:rb + 1, :])
                            rr, rb = rj, 0
                        bc = nrm.tile([128, 512], bf16, name=f"bc{h}",
                                      tag=f"bc{h}")
                        nc.gpsimd.partition_broadcast(
                            bc[:], rr[rb:rb + 1, :])
                        r0 = HALF * j
                        nc.vector.tensor_mul(
                            qnT[h][0:HALF, c0:c1], rT[r0:r0 + HALF, :],
                            bc[r0:r0 + HALF, :])
                        nc.vector.tensor_mul(
                            qnT[h][HALF:HD, c0:c1],
                            rT[64 + r0:64 + r0 + HALF, :],
                            bc[64 + r0:64 + r0 + HALF, :])
                rjk = nrm.tile([1, 512], bf16, name="rjk", tag="rjk")
                nc.vector.tensor_copy(rjk[:], rrowB[32:33, :])
                bck = nrm.tile([64, 512], bf16, name="bck", tag="bck")
                nc.gpsimd.partition_broadcast(bck[:], rjk[:])
                nc.vector.tensor_mul(knT[0:HD, c0:c1], rTk[:], bck[:])

            def attention_head(n, h):
                c0, c1 = n * 512, (n + 1) * 512
                nkt = 4 * n + 4
                yp = psy.tile([HD + 1, 512], f32, name="y", tag="y")
                for i in range(nkt):
                    d = i - 4 * n
                    q0 = 128 * d if d > 0 else 0
                    sp = pss.tile([128, 512], f32, name="s", tag="s")
                    nc.tensor.matmul(
                        sp[:, q0:512],
                        lhsT=knT[:, i * 128:(i + 1) * 128],
                        rhs=qnT[h][:, c0 + q0:c1],
                        start=True, stop=True)
                    ex = expp.tile([128, 512], bf16, name="e", tag="e")
                    nc.scalar.activation(ex[:, q0:512], sp[:, q0:512],
                                         AF.Exp, scale=SCALE)
                    if d >= 0:
                        nc.gpsimd.tensor_mul(
                            ex[:, 128 * d:128 * (d + 1)],
                            ex[:, 128 * d:128 * (d + 1)], mask_sb[:])
                    nc.tensor.matmul(
                        yp[:, q0:512],
                        lhsT=v_all[:, i * VS:i * VS + HD + 1],
                        rhs=ex[:, q0:512],
                        start=(i == 0), stop=(i == nkt - 1))
                rec = smx.tile([1, 512], f32, name="rec", tag="rec")
                nc.vector.reciprocal(rec[:],
                                                 yp[HD:HD + 1, :])
                bc2 = smx.tile([HD, 512], f32, name="bc2", tag="bc2")
                nc.gpsimd.partition_broadcast(bc2[:], rec[:])
                m_, r0 = h // 2, 64 * (h % 2)
                nc.vector.tensor_mul(ynT[m_][r0:r0 + HD, c0:c1],
                                     yp[0:HD, :], bc2[:])

            rg = [[0, 1, 2, 3], [4, 5, 6, 7]]

            def outproj(n):
                import concourse.mybir as mybir
                c0, c1 = n * 512, (n + 1) * 512
                ar_in = dram.tile([C, 512], f32, name=f"arin{n}",
                                  tag=f"arin{n}")
                ar_out = dram.tile([C // 4, 512], f32, name=f"arout{n}",
                                   tag=f"arout{n}")
                osb = smx.tile([128, 8 * 512], f32, name="osb", tag="osb")
                for m8 in range(8):
                    op = pso.tile([128, 512], f32, name="o", tag="o")
                    for k2 in range(2):
                        nc.tensor.matmul(
                            op[:],
                            lhsT=wo_all[:, k2 * C + m8 * 128:
                                        k2 * C + (m8 + 1) * 128],
                            rhs=ynT[k2][:, c0:c1],
                            start=(k2 == 0), stop=(k2 == 1))
                    if n == NQ - 1 and m8 % 2 == 0:
                        nc.scalar.copy(osb[:, m8 * 512:(m8 + 1) * 512],
                                       op[:])
                    else:
                        nc.vector.tensor_copy(
                            osb[:, m8 * 512:(m8 + 1) * 512], op[:])
                    nc.sync.dma_start(ar_in[m8 * 128:(m8 + 1) * 128, :],
                                      osb[:, m8 * 512:(m8 + 1) * 512])
                if SIM_MODE:
                    nc.sync.dma_start(outT[:, c0:c1], ar_in[0:C // 4, :])
                else:
                    nc.gpsimd.collective_compute(
                        "ReduceScatter", mybir.AluOpType.add,
                        replica_groups=rg,
                        ins=[ar_in.opt()], outs=[ar_out.opt()])
                    nc.sync.dma_start(outT[:, c0:c1], ar_out[:])

            # Software pipeline: chunk n+1's projection and rope issue
            # between chunk n's attention heads so the engines overlap.
            def xt_dma(n):
                c0, c1 = n * 512, (n + 1) * 512
                xt = xpool.tile([128, KC * 512], bf16, name="xt", tag="xt")
                nc.sync.dma_start(
                    xt[:].rearrange("p (k t) -> p k t", t=512),
                    xT[:, c0:c1].rearrange("(k p) t -> p k t", p=128))
                return xt

            xt0 = xt_dma(0)
            weight_dmas_front()
            kvr0 = proj_part(0, xt0)
            xt_next = xt_dma(1)
            rts0 = rope_core(0, kvr0)
            norm_finish(0, rts0)
            weight_dmas_back()
            for n in range(NQ):
                attention_head(n, 0)
                if n + 1 < NQ:
                    kvr_next = proj_part(n + 1, xt_next)
                attention_head(n, 1)
                if n + 1 < NQ:
                    rts_next = rope_core(n + 1, kvr_next)
                attention_head(n, 2)
                if n + 2 < NQ:
                    xt_next = xt_dma(n + 2)
                attention_head(n, 3)
                if n + 1 < NQ:
                    norm_finish(n + 1, rts_next)
                outproj(n)

    nc.compile()
    return nc


def _get_nc():
    if "nc" not in _CACHE:
        _CACHE["nc"] = _build()
    return _CACHE["nc"]


def _make_masks():
    p = np.arange(128)[:, None]
    c = np.arange(128)[None, :]
    return (c >= p).astype(np.float32)


def _bf16(a):
    import ml_dtypes
    return np.ascontiguousarray(np.asarray(a).astype(ml_dtypes.bfloat16))


def kernel(x, cos, sin, Wq, Wk, Wv, Wo, _trace=False):
    from concourse.bass_utils import run_bass_kernel_spmd

    nc = _get_nc()
    c_ = np.asarray(cos)[0, :, 0, :].T.astype(np.float32)
    s_ = np.asarray(sin)[0, :, 0, :].T.astype(np.float32)
    cosT = _bf16(np.concatenate([c_, c_, c_, c_], axis=0))
    sinT = _bf16(np.concatenate([s_, s_, s_, s_], axis=0))
    # head-pair permutation of q columns within each 128-wide m-tile:
    # [hA.x1 | hB.x1 | hA.x2 | hB.x2]
    perm = np.zeros(256, dtype=np.int64)
    for mm in range(2):
        base = 128 * mm
        hA, hB = 128 * mm, 128 * mm + 64
        perm[base:base + 32] = hA + np.arange(32)
        perm[base + 32:base + 64] = hB + np.arange(32)
        perm[base + 64:base + 96] = hA + 32 + np.arange(32)
        perm[base + 96:base + 128] = hB + 32 + np.arange(32)
    masks = _bf16(_make_masks())
    eye = _bf16(np.eye(64, dtype=np.float32))
    in_maps = []
    for b in range(DP):
        xT = _bf16(np.asarray(x)[b].T)
        for s in range(TP):
            wkv = np.concatenate([np.asarray(Wk)[64 * s:64 * (s + 1), :],
                                  np.asarray(Wv)[64 * s:64 * (s + 1), :]],
                                 axis=0)
            in_maps.append({
                "xT": xT,
                "cosT": cosT,
                "sinT": sinT,
                "wqT": _bf16(np.asarray(Wq)[256 * s:256 * (s + 1), :]
                             .T[:, perm]),
                "wkvT": _bf16(wkv.T),
                "woT": _bf16(np.asarray(Wo)[:, 256 * s:256 * (s + 1)].T),
                "masks": masks,
                "eye64": eye,
            })
    res = run_bass_kernel_spmd(nc, in_maps, core_ids=list(range(8)),
                               trace=_trace)
    out = np.stack([
        np.concatenate([res.results[c]["outT"] for c in range(4)], axis=0).T,
        np.concatenate([res.results[c]["outT"] for c in range(4, 8)],
                       axis=0).T])
    if _trace:
        _CACHE["last_result"] = res
    return np.ascontiguousarray(out, dtype=np.float32)


# revision 65
# speedup vs baseline: 1.0003x; 1.0003x over previous
"""Distributed GQA causal self-attention (RoPE + RMSNorm QK) for 8 TRN2 cores.

Sharding: DP=2 over batch x TP=4 over KV-head groups.
Core c = 4*b + s handles batch b, kv-group s (1 kv head, 4 q heads).
Per-batch ReduceScatter (replica groups [[0..3],[4..7]]) after the output
projection; the host concatenates the 8 scattered shards.

Layout: features on partitions, tokens on the free axis; the only on-device
transpose is tiny (vT -> v):
  qT = WqT.T @ xT          (256, T)   scoresT = knT.T @ qnT   (kt, qt)
  softmax over kt via exp + matmul with ones-augmented v (row 64 = sums)
  yT = v_aug.T @ expT      (65, qt)   outT = WoT.T @ ynT      (1024, T)
RMS-normed q,k bound scores to |s| <= 8, so exp needs no max subtraction.

All matmul operands are bf16 (incl. the sum-of-squares pass; fp32 rhs costs
4 cycles/row on the PE). Causal structure is exploited at 128-column
granularity inside the diagonal 512-blocks: scores/exp/y all skip the
always-masked query columns. RMS-norm: PE sum-of-squares -> DVE approx
reciprocal -> one batched ACT Sqrt per chunk (sqrt(1/m)), so the scalar
engine only ever alternates Sqrt/Exp once per chunk.
"""

import numpy as np
from contextlib import ExitStack

B, T, C = 2, 2048, 1024
NH, NKV, HD, HALF = 16, 4, 64, 32
G = NH // NKV          # 4 q heads per kv head
TP, DP = 4, 2
KC = C // 128          # 8 contraction tiles
NT = T // 128          # 16 token tiles
NQ = T // 512          # 4 query chunks
SCALE = 1.0 / np.sqrt(HD)
VS = 72                # column stride of packed v blocks (65 used)

_CACHE = {}
SIM_MODE = False


def _build():
    import concourse.bass as bass
    import concourse.bacc as bacc
    import concourse.mybir as mybir
    import concourse.tile as tile

    f32 = mybir.dt.float32
    bf16 = mybir.dt.bfloat16
    AF = mybir.ActivationFunctionType

    nc = bacc.Bacc("TRN2", target_bir_lowering=False, debug=False,
                   num_devices=8)

    xT = nc.dram_tensor("xT", [C, T], bf16, kind="ExternalInput").ap()
    cosT = nc.dram_tensor("cosT", [128, T], bf16, kind="ExternalInput").ap()
    sinT = nc.dram_tensor("sinT", [128, T], bf16, kind="ExternalInput").ap()
    wqT = nc.dram_tensor("wqT", [C, G * HD], bf16, kind="ExternalInput").ap()
    wkvT = nc.dram_tensor("wkvT", [C, 2 * HD], bf16,
                          kind="ExternalInput").ap()
    woT = nc.dram_tensor("woT", [G * HD, C], bf16, kind="ExternalInput").ap()
    masks = nc.dram_tensor("masks", [128, 128], bf16,
                           kind="ExternalInput").ap()
    eye64 = nc.dram_tensor("eye64", [64, 64], bf16, kind="ExternalInput").ap()
    outT = nc.dram_tensor("outT", [C // 4, T], f32, kind="ExternalOutput").ap()

    with tile.TileContext(nc) as tc, ExitStack() as es:
        const = es.enter_context(tc.tile_pool(name="const", bufs=1))
        actp = es.enter_context(tc.tile_pool(name="acts", bufs=1))
        dram = es.enter_context(tc.tile_pool(name="dram", bufs=1, space="DRAM"))

        # ---- persistent weights / tables (merged DMAs) ----
        # DMA order matters: chunk-0 x and Wq go first so the first
        # projection matmuls start as early as possible; Wo last.
        wq_all = const.tile([128, KC * 256], bf16, name="wq", tag="wq")
        wkv_all = const.tile([128, KC * 128], bf16, name="wkv", tag="wkv")
        wo_all = const.tile([128, 2 * C], bf16, name="wo", tag="wo")
        cos_sb = const.tile([128, T], bf16, name="cos", tag="cos")
        sin_sb = const.tile([128, T], bf16, name="sin", tag="sin")
        mask_sb = const.tile([128, 128], bf16, name="mask", tag="mask")
        eye_sb = const.tile([128, 64], bf16, name="eye", tag="eye")

        def weight_dmas_front():
            nc.sync.dma_start(
                wq_all[:].rearrange("p (k m) -> p k m", m=256),
                wqT.rearrange("(k p) m -> p k m", p=128))
            nc.sync.dma_start(
                wkv_all[:].rearrange("p (k m) -> p k m", m=128),
                wkvT.rearrange("(k p) m -> p k m", p=128))
            nc.sync.dma_start(cos_sb[:], cosT)
            nc.sync.dma_start(sin_sb[:], sinT)
            nc.sync.dma_start(eye_sb[64:128, :], eye64)
            nc.sync.dma_start(mask_sb[:], masks)

        def weight_dmas_back():
            nc.sync.dma_start(
                wo_all[:].rearrange("p (k m) -> p k m", m=C),
                woT.rearrange("(k p) m -> p k m", p=128))

        # head-pair selectors for the sum-of-squares matmuls (bf16!)
        onesel = const.tile([128, 2], bf16, name="onesel", tag="onesel")
        nc.any.memset(onesel[:], 0.0)
        nc.any.memset(onesel[0:HALF, 0:1], 1.0)
        nc.any.memset(onesel[HD:HD + HALF, 0:1], 1.0)
        nc.any.memset(onesel[HALF:HD, 1:2], 1.0)
        nc.any.memset(onesel[HD + HALF:128, 1:2], 1.0)
        ones64 = const.tile([64, 1], bf16, name="ones64", tag="ones64")
        nc.any.memset(ones64[:], 1.0)

        # ---- persistent activations ----
        q_raw = [actp.tile([128, T], bf16, name=f"qraw{m}", tag=f"qraw{m}")
                 for m in range(2)]
        qnT = [actp.tile([HD, T], bf16, name=f"qn{h}", tag=f"qn{h}")
               for h in range(G)]
        knT = actp.tile([HD, T], bf16, name="kn", tag="kn")
        v_all = actp.tile([128, NT * VS], bf16, name="vall", tag="vall")
        nc.any.memset(v_all[:, HD::VS], 1.0)   # ones column of each v block
        ynT = [actp.tile([128, T], bf16, name=f"yn{m}", tag=f"yn{m}")
               for m in range(2)]
        # rms staging: engine APs need partition bases in {0,32,64}, so
        # heads 0-2 sit at rows {0,32,64} of tile A and head 3 / k at rows
        # {0,32} of tile B; the gaps are memset once so the batched Sqrts
        # read defined values.
        minvA = actp.tile([65, 512], f32, name="minvA", tag="minvA")
        nc.any.memset(minvA[:], 1.0)
        minvB = actp.tile([33, 512], f32, name="minvB", tag="minvB")
        nc.any.memset(minvB[:], 1.0)
        rrowA = actp.tile([65, 512], bf16, name="rrowA", tag="rrowA")
        rrowB = actp.tile([33, 512], bf16, name="rrowB", tag="rrowB")

        with tc.tile_pool(name="xTp", bufs=2) as xpool, \
             tc.tile_pool(name="kvp", bufs=3) as kvpool, \
             tc.tile_pool(name="rp", bufs=3) as rp, \
             tc.tile_pool(name="nrm", bufs=3) as nrm, \
             tc.tile_pool(name="expp", bufs=6) as expp, \
             tc.tile_pool(name="smx", bufs=3) as smx, \
             tc.tile_pool(name="ps1", bufs=1, space="PSUM") as ps1, \
             tc.tile_pool(name="pss", bufs=3, space="PSUM") as pss, \
             tc.tile_pool(name="psy", bufs=1, space="PSUM") as psy, \
             tc.tile_pool(name="pso", bufs=2, space="PSUM") as pso:

            def rope6(src, W, c0, c1, rT, tP, t0=None, t1=None):
                """rT[0:W]=x1*cos+x2*sin ; rT[W:2W]=x2*cos-x1*sin.

                cos_sb/sin_sb are stacked x4 so x2 (at partition base W)
                multiplies against same-base cos/sin rows — no align copy.
                Product outputs land at whatever base the add/sub needs
                (out base may differ from input base; input bases must
                match each other)."""
                if t0 is None:
                    t0, t1 = c0, c1
                x1 = src[0:W, c0:c1]
                x2 = src[W:2 * W, c0:c1]
                cs, sn = cos_sb[0:W, t0:t1], sin_sb[0:W, t0:t1]
                cs2, sn2 = cos_sb[W:2 * W, t0:t1], sin_sb[W:2 * W, t0:t1]
                nc.vector.tensor_mul(rT[0:W, :], x1, cs)
                nc.vector.tensor_mul(tP[0:W, :], x2, sn2)
                nc.vector.tensor_add(rT[0:W, :], rT[0:W, :], tP[0:W, :])
                nc.vector.tensor_mul(rT[W:2 * W, :], x2, cs2)
                nc.vector.tensor_mul(tP[W:2 * W, :], x1, sn)
                nc.vector.tensor_sub(rT[W:2 * W, :], rT[W:2 * W, :],
                                     tP[W:2 * W, :])

            def proj_part(n, xt):
                c0, c1 = n * 512, (n + 1) * 512
                # q projection (2 m-tiles) + packed kv projection
                for m in range(2):
                    qp = ps1.tile([128, 512], f32, name="pq", tag="pq")
                    for k in range(KC):
                        nc.tensor.matmul(
                            qp[:],
                            lhsT=wq_all[:, k * 256 + m * 128:
                                        k * 256 + (m + 1) * 128],
                            rhs=xt[:, k * 512:(k + 1) * 512],
                            start=(k == 0), stop=(k == KC - 1))
                    nc.scalar.copy(q_raw[m][:, c0:c1], qp[:])
                kvp = ps1.tile([128, 512], f32, name="pkv", tag="pq")
                for k in range(KC):
                    nc.tensor.matmul(
                        kvp[:], lhsT=wkv_all[:, k * 128:(k + 1) * 128],
                        rhs=xt[:, k * 512:(k + 1) * 512],
                        start=(k == 0), stop=(k == KC - 1))
                kvr = kvpool.tile([128, 512], bf16, name="kvr", tag="kvr")
                nc.scalar.copy(kvr[:], kvp[:])

                # v transpose: 4 token tiles -> one psum bank -> v_all blocks
                tp = ps1.tile([128, 4 * HD], bf16, name="tp", tag="ss")
                for it in range(4):
                    nc.tensor.matmul(
                        tp[:, it * HD:(it + 1) * HD],
                        lhsT=kvr[HD:128, it * 128:(it + 1) * 128],
                        rhs=eye_sb[64:128, :], is_transpose=True,
                        skip_group_check=(it > 0))
                nc.vector.tensor_copy(
                    v_all[:].rearrange("p (i s) -> p i s", s=VS)
                    [:, 4 * n:4 * n + 4, 0:HD],
                    tp[:].rearrange("p (i c) -> p i c", c=HD))
                return kvr

            def rope_core(n, kvr):
                c0, c1 = n * 512, (n + 1) * 512
                # rope (2 q pairs + k) + sum-of-squares -> rms factors
                sq = []
                for m in range(2):
                    rT = rp.tile([128, 512], bf16, name=f"rq{m}",
                                 tag=f"rq{m}")
                    tP = rp.tile([128, 512], bf16, name=f"tq{m}",
                                 tag=f"tq{m}")
                    rope6(q_raw[m], HD, c0, c1, rT, tP)
                    s = rp.tile([128, 512], bf16, name=f"sq{m}",
                                tag=f"sq{m}")
                    nc.vector.tensor_mul(s[:], rT[:], rT[:])
                    sq.append((rT, s))
                rTk = rp.tile([64, 512], bf16, name="rk", tag="rk")
                tPk = rp.tile([64, 512], bf16, name="tk", tag="tk")
                rope6(kvr, HALF, 0, 512, rTk, tPk, t0=c0, t1=c1)
                sk = rp.tile([64, 512], bf16, name="sk", tag="sk")
                nc.vector.tensor_mul(sk[:], rTk[:], rTk[:])

                # ss matmuls (M=1 each, psum rows at {0,32,64});
                # DVE approx recip -> minv; two batched Sqrts
                ssqA = ps1.tile([65, 512], f32, name="ssA", tag="ss")
                ssqB = ps1.tile([65, 512], f32, name="ssB", tag="ss")
                # (tile, row) per head 0..3, then k
                slots = [(ssqA, minvA, 0), (ssqA, minvA, 32),
                         (ssqA, minvA, 64), (ssqB, minvB, 0)]
                for h in range(4):
                    m, j = h // 2, h % 2
                    ssq, mv, b = slots[h]
                    nc.tensor.matmul(ssq[b:b + 1, :],
                                     lhsT=onesel[:, j:j + 1],
                                     rhs=sq[m][1][:],
                                     start=True, stop=True,
                                     skip_group_check=(ssq is ssqA
                                                       and b > 0))
                    nc.vector.reciprocal(
                        mv[b:b + 1, :], ssq[b:b + 1, :])
                nc.tensor.matmul(ssqB[32:33, :], lhsT=ones64[:], rhs=sk[:],
                                 start=True, stop=True,
                                 skip_group_check=True)
                nc.vector.reciprocal(minvB[32:33, :],
                                                 ssqB[32:33, :])
                # rrow = sqrt(64 / ss) = rsqrt(mean(x^2))
                nc.scalar.activation(rrowA[:], minvA[:], AF.Sqrt,
                                     scale=float(HD))
                nc.scalar.activation(rrowB[:], minvB[:], AF.Sqrt,
                                     scale=float(HD))
                return sq, rTk

            def norm_finish(n, rts):
                c0, c1 = n * 512, (n + 1) * 512
                sq, rTk = rts
                # normalize: qnT[h] and knT
                # partition_broadcast inputs MUST sit at partition 0 of a
                # tile (offset inputs silently mis-lower in walrus), so rows
                # at base 32/64 are staged through base-0 copies first.
                rsrc = [(rrowA, 0), (rrowA, 32), (rrowA, 64), (rrowB, 0)]
                for m in range(2):
                    rT = sq[m][0]
                    for j in range(2):
                        h = 2 * m + j
                        rr, rb = rsrc[h]
                        if rb > 0:
                            rj = nrm.tile([1, 512], bf16, name=f"rj{h}",
                                          tag=f"rj{h}")
                            nc.vector.tensor_copy(rj[:], rr[rb:rb + 1, :])
                            rr, rb = rj, 0
                        bc = nrm.tile([128, 512], bf16, name=f"bc{h}",
                                      tag=f"bc{h}")
                        nc.gpsimd.partition_broadcast(
                            bc[:], rr[rb:rb + 1, :])
                        r0 = HALF * j
                        nc.vector.tensor_mul(
                            qnT[h][0:HALF, c0:c1], rT[r0:r0 + HALF, :],
                            bc[r0:r0 + HALF, :])
                        nc.vector.tensor_mul(
                            qnT[h][HALF:HD, c0:c1],
                            rT[64 + r0:64 + r0 + HALF, :],
                            bc[64 + r0:64 + r0 + HALF, :])
                rjk = nrm.tile([1, 512], bf16, name="rjk", tag="rjk")
                nc.vector.tensor_copy(rjk[:], rrowB[32:33, :])
                bck = nrm.tile([64, 512], bf16, name="bck", tag="bck")
                nc.gpsimd.partition_broadcast(bck[:], rjk[:])
                nc.vector.tensor_mul(knT[0:HD, c0:c1], rTk[:], bck[:])

            def attention_head(n, h):
                c0, c1 = n * 512, (n + 1) * 512
                nkt = 4 * n + 4
                yp = psy.tile([HD + 1, 512], f32, name="y", tag="y")
                for i in range(nkt):
                    d = i - 4 * n
                    q0 = 128 * d if d > 0 else 0
                    sp = pss.tile([128, 512], f32, name="s", tag="s")
                    nc.tensor.matmul(
                        sp[:, q0:512],
                        lhsT=knT[:, i * 128:(i + 1) * 128],
                        rhs=qnT[h][:, c0 + q0:c1],
                        start=True, stop=True)
                    ex = expp.tile([128, 512], bf16, name="e", tag="e")
                    nc.scalar.activation(ex[:, q0:512], sp[:, q0:512],
                                         AF.Exp, scale=SCALE)
                    if d >= 0:
                        nc.gpsimd.tensor_mul(
                            ex[:, 128 * d:128 * (d + 1)],
                            ex[:, 128 * d:128 * (d + 1)], mask_sb[:])
                    nc.tensor.matmul(
                        yp[:, q0:512],
                        lhsT=v_all[:, i * VS:i * VS + HD + 1],
                        rhs=ex[:, q0:512],
                        start=(i == 0), stop=(i == nkt - 1))
                rec = smx.tile([1, 512], f32, name="rec", tag="rec")
                nc.vector.reciprocal(rec[:],
                                                 yp[HD:HD + 1, :])
                bc2 = smx.tile([HD, 512], f32, name="bc2", tag="bc2")
                nc.gpsimd.partition_broadcast(bc2[:], rec[:])
                m_, r0 = h // 2, 64 * (h % 2)
                nc.vector.tensor_mul(ynT[m_][r0:r0 + HD, c0:c1],
                                     yp[0:HD, :], bc2[:])

            rg = [[0, 1, 2, 3], [4, 5, 6, 7]]

            def outproj(n):
                import concourse.mybir as mybir
                c0, c1 = n * 512, (n + 1) * 512
                ar_in = dram.tile([C, 512], f32, name=f"arin{n}",
                                  tag=f"arin{n}")
                ar_out = dram.tile([C // 4, 512], f32, name=f"arout{n}",
                                   tag=f"arout{n}")
                osb = smx.tile([128, 8 * 512], f32, name="osb", tag="osb")
                for m8 in range(8):
                    op = pso.tile([128, 512], f32, name="o", tag="o")
                    for k2 in range(2):
                        nc.tensor.matmul(
                            op[:],
                            lhsT=wo_all[:, k2 * C + m8 * 128:
                                        k2 * C + (m8 + 1) * 128],
                            rhs=ynT[k2][:, c0:c1],
                            start=(k2 == 0), stop=(k2 == 1))
                    if n == NQ - 1 and m8 % 2 == 0:
                        nc.scalar.copy(osb[:, m8 * 512:(m8 + 1) * 512],
                                       op[:])
                    else:
                        nc.vector.tensor_copy(
                            osb[:, m8 * 512:(m8 + 1) * 512], op[:])
                    nc.sync.dma_start(ar_in[m8 * 128:(m8 + 1) * 128, :],
                                      osb[:, m8 * 512:(m8 + 1) * 512])
                if SIM_MODE:
                    nc.sync.dma_start(outT[:, c0:c1], ar_in[0:C // 4, :])
                else:
                    nc.gpsimd.collective_compute(
                        "ReduceScatter", mybir.AluOpType.add,
                        replica_groups=rg,
                        ins=[ar_in.opt()], outs=[ar_out.opt()])
                    nc.sync.dma_start(outT[:, c0:c1], ar_out[:])

            # Software pipeline: chunk n+1's projection and rope issue
            # between chunk n's attention heads so the engines overlap.
            def xt_dma(n, split=False):
                c0, c1 = n * 512, (n + 1) * 512
                xt = xpool.tile([128, KC * 512], bf16, name="xt", tag="xt")
                if split:
                    # prologue only: two half-DMAs so the first projection
                    # matmuls start after half the transfer
                    for h2 in range(2):
                        nc.sync.dma_start(
                            xt[:, h2 * 2048:(h2 + 1) * 2048]
                            .rearrange("p (k t) -> p k t", t=512),
                            xT[h2 * 512:(h2 + 1) * 512, c0:c1]
                            .rearrange("(k p) t -> p k t", p=128))
                else:
                    nc.sync.dma_start(
                        xt[:].rearrange("p (k t) -> p k t", t=512),
                        xT[:, c0:c1].rearrange("(k p) t -> p k t", p=128))
                return xt

            xt0 = xt_dma(0, split=True)
            weight_dmas_front()
            kvr0 = proj_part(0, xt0)
            xt_next = xt_dma(1)
            rts0 = rope_core(0, kvr0)
            norm_finish(0, rts0)
            weight_dmas_back()
            for n in range(NQ):
                attention_head(n, 0)
                if n + 1 < NQ:
                    kvr_next = proj_part(n + 1, xt_next)
                attention_head(n, 1)
                if n + 1 < NQ:
                    rts_next = rope_core(n + 1, kvr_next)
                attention_head(n, 2)
                if n + 2 < NQ:
                    xt_next = xt_dma(n + 2)
                attention_head(n, 3)
                if n + 1 < NQ:
                    norm_finish(n + 1, rts_next)
                outproj(n)

    nc.compile()
    return nc


def _get_nc():
    if "nc" not in _CACHE:
        _CACHE["nc"] = _build()
    return _CACHE["nc"]


def _make_masks():
    p = np.arange(128)[:, None]
    c = np.arange(128)[None, :]
    return (c >= p).astype(np.float32)


def _bf16(a):
    import ml_dtypes
    return np.ascontiguousarray(np.asarray(a).astype(ml_dtypes.bfloat16))


def kernel(x, cos, sin, Wq, Wk, Wv, Wo, _trace=False):
    from concourse.bass_utils import run_bass_kernel_spmd

    nc = _get_nc()
    c_ = np.asarray(cos)[0, :, 0, :].T.astype(np.float32)
    s_ = np.asarray(sin)[0, :, 0, :].T.astype(np.float32)
    cosT = _bf16(np.concatenate([c_, c_, c_, c_], axis=0))
    sinT = _bf16(np.concatenate([s_, s_, s_, s_], axis=0))
    # head-pair permutation of q columns within each 128-wide m-tile:
    # [hA.x1 | hB.x1 | hA.x2 | hB.x2]
    perm = np.zeros(256, dtype=np.int64)
    for mm in range(2):
        base = 128 * mm
        hA, hB = 128 * mm, 128 * mm + 64
        perm[base:base + 32] = hA + np.arange(32)
        perm[base + 32:base + 64] = hB + np.arange(32)
        perm[base + 64:base + 96] = hA + 32 + np.arange(32)
        perm[base + 96:base + 128] = hB + 32 + np.arange(32)
    masks = _bf16(_make_masks())
    eye = _bf16(np.eye(64, dtype=np.float32))
    in_maps = []
    for b in range(DP):
        xT = _bf16(np.asarray(x)[b].T)
        for s in range(TP):
            wkv = np.concatenate([np.asarray(Wk)[64 * s:64 * (s + 1), :],
                                  np.asarray(Wv)[64 * s:64 * (s + 1), :]],
                                 axis=0)
            in_maps.append({
                "xT": xT,
                "cosT": cosT,
                "sinT": sinT,
                "wqT": _bf16(np.asarray(Wq)[256 * s:256 * (s + 1), :]
                             .T[:, perm]),
                "wkvT": _bf16(wkv.T),
                "woT": _bf16(np.asarray(Wo)[:, 256 * s:256 * (s + 1)].T),
                "masks": masks,
                "eye64": eye,
            })
    res = run_bass_kernel_spmd(nc, in_maps, core_ids=list(range(8)),
                               trace=_trace)
    out = np.stack([
        np.concatenate([res.results[c]["outT"] for c in range(4)], axis=0).T,
        np.concatenate([res.results[c]["outT"] for c in range(4, 8)],
                       axis=0).T])
    if _trace:
        _CACHE["last_result"] = res
    return np.ascontiguousarray(out, dtype=np.float32)


# revision 66
# speedup vs baseline: 1.0267x; 1.0264x over previous
"""Distributed GQA causal self-attention (RoPE + RMSNorm QK) for 8 TRN2 cores.

Sharding: DP=2 over batch x TP=4 over KV-head groups.
Core c = 4*b + s handles batch b, kv-group s (1 kv head, 4 q heads).
Per-batch ReduceScatter (replica groups [[0..3],[4..7]]) after the output
projection; the host concatenates the 8 scattered shards.

Layout: features on partitions, tokens on the free axis; the only on-device
transpose is tiny (vT -> v):
  qT = WqT.T @ xT          (256, T)   scoresT = knT.T @ qnT   (kt, qt)
  softmax over kt via exp + matmul with ones-augmented v (row 64 = sums)
  yT = v_aug.T @ expT      (65, qt)   outT = WoT.T @ ynT      (1024, T)
RMS-normed q,k bound scores to |s| <= 8, so exp needs no max subtraction.

All matmul operands are bf16 (incl. the sum-of-squares pass; fp32 rhs costs
4 cycles/row on the PE). Causal structure is exploited at 128-column
granularity inside the diagonal 512-blocks: scores/exp/y all skip the
always-masked query columns. RMS-norm: PE sum-of-squares -> DVE approx
reciprocal -> one batched ACT Sqrt per chunk (sqrt(1/m)), so the scalar
engine only ever alternates Sqrt/Exp once per chunk.
"""

import numpy as np
from contextlib import ExitStack

B, T, C = 2, 2048, 1024
NH, NKV, HD, HALF = 16, 4, 64, 32
G = NH // NKV          # 4 q heads per kv head
TP, DP = 4, 2
KC = C // 128          # 8 contraction tiles
NT = T // 128          # 16 token tiles
NQ = T // 512          # 4 query chunks
SCALE = 1.0 / np.sqrt(HD)
VS = 72                # column stride of packed v blocks (65 used)

_CACHE = {}
SIM_MODE = False


def _build():
    import concourse.bass as bass
    import concourse.bacc as bacc
    import concourse.mybir as mybir
    import concourse.tile as tile

    f32 = mybir.dt.float32
    bf16 = mybir.dt.bfloat16
    AF = mybir.ActivationFunctionType

    nc = bacc.Bacc("TRN2", target_bir_lowering=False, debug=False,
                   num_devices=8)

    xT = nc.dram_tensor("xT", [C, T], bf16, kind="ExternalInput").ap()
    cosT = nc.dram_tensor("cosT", [128, T], bf16, kind="ExternalInput").ap()
    sinT = nc.dram_tensor("sinT", [128, T], bf16, kind="ExternalInput").ap()
    wqT = nc.dram_tensor("wqT", [C, G * HD], bf16, kind="ExternalInput").ap()
    wkvT = nc.dram_tensor("wkvT", [C, 2 * HD], bf16,
                          kind="ExternalInput").ap()
    woT = nc.dram_tensor("woT", [G * HD, C], bf16, kind="ExternalInput").ap()
    masks = nc.dram_tensor("masks", [128, 128], bf16,
                           kind="ExternalInput").ap()
    eye64 = nc.dram_tensor("eye64", [64, 64], bf16, kind="ExternalInput").ap()
    outT = nc.dram_tensor("outT", [C // 4, T], f32, kind="ExternalOutput").ap()

    with tile.TileContext(nc) as tc, ExitStack() as es:
        const = es.enter_context(tc.tile_pool(name="const", bufs=1))
        actp = es.enter_context(tc.tile_pool(name="acts", bufs=1))
        dram = es.enter_context(tc.tile_pool(name="dram", bufs=1, space="DRAM"))

        # ---- persistent weights / tables (merged DMAs) ----
        # DMA order matters: chunk-0 x and Wq go first so the first
        # projection matmuls start as early as possible; Wo last.
        wq_all = const.tile([128, KC * 256], bf16, name="wq", tag="wq")
        wkv_all = const.tile([128, KC * 128], bf16, name="wkv", tag="wkv")
        wo_all = const.tile([128, 2 * C], bf16, name="wo", tag="wo")
        cos_sb = const.tile([128, T], bf16, name="cos", tag="cos")
        sin_sb = const.tile([128, T], bf16, name="sin", tag="sin")
        mask_sb = const.tile([128, 128], bf16, name="mask", tag="mask")
        eye_sb = const.tile([128, 64], bf16, name="eye", tag="eye")

        def weight_dmas_front():
            nc.sync.dma_start(
                wq_all[:].rearrange("p (k m) -> p k m", m=256),
                wqT.rearrange("(k p) m -> p k m", p=128))
            nc.sync.dma_start(
                wkv_all[:].rearrange("p (k m) -> p k m", m=128),
                wkvT.rearrange("(k p) m -> p k m", p=128))
            nc.sync.dma_start(cos_sb[:], cosT)
            nc.sync.dma_start(sin_sb[:], sinT)
            nc.sync.dma_start(eye_sb[64:128, :], eye64)
            nc.sync.dma_start(mask_sb[:], masks)

        def weight_dmas_back():
            nc.sync.dma_start(
                wo_all[:].rearrange("p (k m) -> p k m", m=C),
                woT.rearrange("(k p) m -> p k m", p=128))

        # head-pair selectors for the sum-of-squares matmuls (bf16!)
        onesel = const.tile([128, 2], bf16, name="onesel", tag="onesel")
        nc.any.memset(onesel[:], 0.0)
        nc.any.memset(onesel[0:HALF, 0:1], 1.0)
        nc.any.memset(onesel[HD:HD + HALF, 0:1], 1.0)
        nc.any.memset(onesel[HALF:HD, 1:2], 1.0)
        nc.any.memset(onesel[HD + HALF:128, 1:2], 1.0)
        ones64 = const.tile([64, 1], bf16, name="ones64", tag="ones64")
        nc.any.memset(ones64[:], 1.0)

        # ---- persistent activations ----
        q_raw = [actp.tile([128, T], bf16, name=f"qraw{m}", tag=f"qraw{m}")
                 for m in range(2)]
        qnT = [actp.tile([HD, T], bf16, name=f"qn{h}", tag=f"qn{h}")
               for h in range(G)]
        knT = actp.tile([HD, T], bf16, name="kn", tag="kn")
        v_all = actp.tile([128, NT * VS], bf16, name="vall", tag="vall")
        nc.any.memset(v_all[:, HD::VS], 1.0)   # ones column of each v block
        ynT = [actp.tile([128, T], bf16, name=f"yn{m}", tag=f"yn{m}")
               for m in range(2)]
        # rms staging: engine APs need partition bases in {0,32,64}, so
        # heads 0-2 sit at rows {0,32,64} of tile A and head 3 / k at rows
        # {0,32} of tile B; the gaps are memset once so the batched Sqrts
        # read defined values.
        minvA = actp.tile([65, 512], f32, name="minvA", tag="minvA")
        nc.any.memset(minvA[:], 1.0)
        minvB = actp.tile([33, 512], f32, name="minvB", tag="minvB")
        nc.any.memset(minvB[:], 1.0)
        rrowA = actp.tile([65, 512], bf16, name="rrowA", tag="rrowA")
        rrowB = actp.tile([33, 512], bf16, name="rrowB", tag="rrowB")

        with tc.tile_pool(name="xTp", bufs=2) as xpool, \
             tc.tile_pool(name="kvp", bufs=3) as kvpool, \
             tc.tile_pool(name="rp", bufs=3) as rp, \
             tc.tile_pool(name="nrm", bufs=3) as nrm, \
             tc.tile_pool(name="expp", bufs=6) as expp, \
             tc.tile_pool(name="smx", bufs=3) as smx, \
             tc.tile_pool(name="ps1", bufs=1, space="PSUM") as ps1, \
             tc.tile_pool(name="pss", bufs=3, space="PSUM") as pss, \
             tc.tile_pool(name="psy", bufs=1, space="PSUM") as psy, \
             tc.tile_pool(name="pso", bufs=2, space="PSUM") as pso:

            def rope6(src, W, c0, c1, rT, tP, t0=None, t1=None):
                """rT[0:W]=x1*cos+x2*sin ; rT[W:2W]=x2*cos-x1*sin.

                cos_sb/sin_sb are stacked x4 so x2 (at partition base W)
                multiplies against same-base cos/sin rows — no align copy.
                Product outputs land at whatever base the add/sub needs
                (out base may differ from input base; input bases must
                match each other)."""
                if t0 is None:
                    t0, t1 = c0, c1
                x1 = src[0:W, c0:c1]
                x2 = src[W:2 * W, c0:c1]
                cs, sn = cos_sb[0:W, t0:t1], sin_sb[0:W, t0:t1]
                cs2, sn2 = cos_sb[W:2 * W, t0:t1], sin_sb[W:2 * W, t0:t1]
                nc.vector.tensor_mul(rT[0:W, :], x1, cs)
                nc.vector.tensor_mul(tP[0:W, :], x2, sn2)
                nc.vector.tensor_add(rT[0:W, :], rT[0:W, :], tP[0:W, :])
                nc.vector.tensor_mul(rT[W:2 * W, :], x2, cs2)
                nc.vector.tensor_mul(tP[W:2 * W, :], x1, sn)
                nc.vector.tensor_sub(rT[W:2 * W, :], rT[W:2 * W, :],
                                     tP[W:2 * W, :])

            def proj_part(n, xt):
                c0, c1 = n * 512, (n + 1) * 512
                # q projection (2 m-tiles) + packed kv projection
                for m in range(2):
                    qp = ps1.tile([128, 512], f32, name="pq", tag="pq")
                    for k in range(KC):
                        nc.tensor.matmul(
                            qp[:],
                            lhsT=wq_all[:, k * 256 + m * 128:
                                        k * 256 + (m + 1) * 128],
                            rhs=xt[:, k * 512:(k + 1) * 512],
                            start=(k == 0), stop=(k == KC - 1))
                    nc.scalar.copy(q_raw[m][:, c0:c1], qp[:])
                kvp = ps1.tile([128, 512], f32, name="pkv", tag="pq")
                for k in range(KC):
                    nc.tensor.matmul(
                        kvp[:], lhsT=wkv_all[:, k * 128:(k + 1) * 128],
                        rhs=xt[:, k * 512:(k + 1) * 512],
                        start=(k == 0), stop=(k == KC - 1))
                kvr = kvpool.tile([128, 512], bf16, name="kvr", tag="kvr")
                nc.scalar.copy(kvr[:], kvp[:])

                # v transpose: 4 token tiles -> one psum bank -> v_all blocks
                tp = ps1.tile([128, 4 * HD], bf16, name="tp", tag="ss")
                for it in range(4):
                    nc.tensor.matmul(
                        tp[:, it * HD:(it + 1) * HD],
                        lhsT=kvr[HD:128, it * 128:(it + 1) * 128],
                        rhs=eye_sb[64:128, :], is_transpose=True,
                        skip_group_check=(it > 0))
                nc.vector.tensor_copy(
                    v_all[:].rearrange("p (i s) -> p i s", s=VS)
                    [:, 4 * n:4 * n + 4, 0:HD],
                    tp[:].rearrange("p (i c) -> p i c", c=HD))
                return kvr

            def rope_core(n, kvr):
                c0, c1 = n * 512, (n + 1) * 512
                # rope (2 q pairs + k) + sum-of-squares -> rms factors
                sq = []
                for m in range(2):
                    rT = rp.tile([128, 512], bf16, name=f"rq{m}",
                                 tag=f"rq{m}")
                    tP = rp.tile([128, 512], bf16, name=f"tq{m}",
                                 tag=f"tq{m}")
                    rope6(q_raw[m], HD, c0, c1, rT, tP)
                    s = rp.tile([128, 512], bf16, name=f"sq{m}",
                                tag=f"sq{m}")
                    nc.vector.tensor_mul(s[:], rT[:], rT[:])
                    sq.append((rT, s))
                rTk = rp.tile([64, 512], bf16, name="rk", tag="rk")
                tPk = rp.tile([64, 512], bf16, name="tk", tag="tk")
                rope6(kvr, HALF, 0, 512, rTk, tPk, t0=c0, t1=c1)
                sk = rp.tile([64, 512], bf16, name="sk", tag="sk")
                nc.vector.tensor_mul(sk[:], rTk[:], rTk[:])

                # ss matmuls (M=1 each, psum rows at {0,32,64});
                # DVE approx recip -> minv; two batched Sqrts
                ssqA = ps1.tile([65, 512], f32, name="ssA", tag="ss")
                ssqB = ps1.tile([65, 512], f32, name="ssB", tag="ss")
                # (tile, row) per head 0..3, then k
                slots = [(ssqA, minvA, 0), (ssqA, minvA, 32),
                         (ssqA, minvA, 64), (ssqB, minvB, 0)]
                for h in range(4):
                    m, j = h // 2, h % 2
                    ssq, mv, b = slots[h]
                    nc.tensor.matmul(ssq[b:b + 1, :],
                                     lhsT=onesel[:, j:j + 1],
                                     rhs=sq[m][1][:],
                                     start=True, stop=True,
                                     skip_group_check=(ssq is ssqA
                                                       and b > 0))
                    nc.vector.reciprocal(
                        mv[b:b + 1, :], ssq[b:b + 1, :])
                nc.tensor.matmul(ssqB[32:33, :], lhsT=ones64[:], rhs=sk[:],
                                 start=True, stop=True,
                                 skip_group_check=True)
                nc.vector.reciprocal(minvB[32:33, :],
                                                 ssqB[32:33, :])
                # rrow = sqrt(64 / ss) = rsqrt(mean(x^2))
                nc.scalar.activation(rrowA[:], minvA[:], AF.Sqrt,
                                     scale=float(HD))
                nc.scalar.activation(rrowB[:], minvB[:], AF.Sqrt,
                                     scale=float(HD))
                return sq, rTk

            def norm_finish(n, rts):
                c0, c1 = n * 512, (n + 1) * 512
                sq, rTk = rts
                # normalize: qnT[h] and knT
                # partition_broadcast inputs MUST sit at partition 0 of a
                # tile (offset inputs silently mis-lower in walrus), so rows
                # at base 32/64 are staged through base-0 copies first.
                rsrc = [(rrowA, 0), (rrowA, 32), (rrowA, 64), (rrowB, 0)]
                for m in range(2):
                    rT = sq[m][0]
                    for j in range(2):
                        h = 2 * m + j
                        rr, rb = rsrc[h]
                        if rb > 0:
                            rj = nrm.tile([1, 512], bf16, name=f"rj{h}",
                                          tag=f"rj{h}")
                            nc.vector.tensor_copy(rj[:], rr[rb:rb + 1, :])
                            rr, rb = rj, 0
                        bc = nrm.tile([128, 512], bf16, name=f"bc{h}",
                                      tag=f"bc{h}")
                        nc.gpsimd.partition_broadcast(
                            bc[:], rr[rb:rb + 1, :])
                        r0 = HALF * j
                        nc.vector.tensor_mul(
                            qnT[h][0:HALF, c0:c1], rT[r0:r0 + HALF, :],
                            bc[r0:r0 + HALF, :])
                        nc.vector.tensor_mul(
                            qnT[h][HALF:HD, c0:c1],
                            rT[64 + r0:64 + r0 + HALF, :],
                            bc[64 + r0:64 + r0 + HALF, :])
                rjk = nrm.tile([1, 512], bf16, name="rjk", tag="rjk")
                nc.vector.tensor_copy(rjk[:], rrowB[32:33, :])
                bck = nrm.tile([64, 512], bf16, name="bck", tag="bck")
                nc.gpsimd.partition_broadcast(bck[:], rjk[:])
                nc.vector.tensor_mul(knT[0:HD, c0:c1], rTk[:], bck[:])

            def attention_head(n, h):
                c0, c1 = n * 512, (n + 1) * 512
                nkt = 4 * n + 4
                yp = psy.tile([HD + 1, 512], f32, name="y", tag="y")
                for i in range(nkt):
                    d = i - 4 * n
                    q0 = 128 * d if d > 0 else 0
                    sp = pss.tile([128, 512], f32, name="s", tag="s")
                    nc.tensor.matmul(
                        sp[:, q0:512],
                        lhsT=knT[:, i * 128:(i + 1) * 128],
                        rhs=qnT[h][:, c0 + q0:c1],
                        start=True, stop=True)
                    ex = expp.tile([128, 512], bf16, name="e", tag="e")
                    nc.scalar.activation(ex[:, q0:512], sp[:, q0:512],
                                         AF.Exp, scale=SCALE)
                    if d >= 0:
                        meng = nc.vector if n >= 2 else nc.gpsimd
                        meng.tensor_mul(
                            ex[:, 128 * d:128 * (d + 1)],
                            ex[:, 128 * d:128 * (d + 1)], mask_sb[:])
                    nc.tensor.matmul(
                        yp[:, q0:512],
                        lhsT=v_all[:, i * VS:i * VS + HD + 1],
                        rhs=ex[:, q0:512],
                        start=(i == 0), stop=(i == nkt - 1))
                rec = smx.tile([1, 512], f32, name="rec", tag="rec")
                nc.vector.reciprocal(rec[:],
                                                 yp[HD:HD + 1, :])
                bc2 = smx.tile([HD, 512], f32, name="bc2", tag="bc2")
                nc.gpsimd.partition_broadcast(bc2[:], rec[:])
                m_, r0 = h // 2, 64 * (h % 2)
                nc.vector.tensor_mul(ynT[m_][r0:r0 + HD, c0:c1],
                                     yp[0:HD, :], bc2[:])

            rg = [[0, 1, 2, 3], [4, 5, 6, 7]]

            def outproj(n):
                import concourse.mybir as mybir
                c0, c1 = n * 512, (n + 1) * 512
                ar_in = dram.tile([C, 512], f32, name=f"arin{n}",
                                  tag=f"arin{n}")
                ar_out = dram.tile([C // 4, 512], f32, name=f"arout{n}",
                                   tag=f"arout{n}")
                osb = smx.tile([128, 8 * 512], f32, name="osb", tag="osb")
                for m8 in range(8):
                    op = pso.tile([128, 512], f32, name="o", tag="o")
                    for k2 in range(2):
                        nc.tensor.matmul(
                            op[:],
                            lhsT=wo_all[:, k2 * C + m8 * 128:
                                        k2 * C + (m8 + 1) * 128],
                            rhs=ynT[k2][:, c0:c1],
                            start=(k2 == 0), stop=(k2 == 1))
                    if n == NQ - 1 and m8 % 2 == 0:
                        nc.scalar.copy(osb[:, m8 * 512:(m8 + 1) * 512],
                                       op[:])
                    else:
                        nc.vector.tensor_copy(
                            osb[:, m8 * 512:(m8 + 1) * 512], op[:])
                    nc.sync.dma_start(ar_in[m8 * 128:(m8 + 1) * 128, :],
                                      osb[:, m8 * 512:(m8 + 1) * 512])
                if SIM_MODE:
                    nc.sync.dma_start(outT[:, c0:c1], ar_in[0:C // 4, :])
                else:
                    nc.gpsimd.collective_compute(
                        "ReduceScatter", mybir.AluOpType.add,
                        replica_groups=rg,
                        ins=[ar_in.opt()], outs=[ar_out.opt()])
                    nc.sync.dma_start(outT[:, c0:c1], ar_out[:])

            # Software pipeline: chunk n+1's projection and rope issue
            # between chunk n's attention heads so the engines overlap.
            def xt_dma(n, split=False):
                c0, c1 = n * 512, (n + 1) * 512
                xt = xpool.tile([128, KC * 512], bf16, name="xt", tag="xt")
                if split:
                    # prologue only: two half-DMAs so the first projection
                    # matmuls start after half the transfer
                    for h2 in range(2):
                        nc.sync.dma_start(
                            xt[:, h2 * 2048:(h2 + 1) * 2048]
                            .rearrange("p (k t) -> p k t", t=512),
                            xT[h2 * 512:(h2 + 1) * 512, c0:c1]
                            .rearrange("(k p) t -> p k t", p=128))
                else:
                    nc.sync.dma_start(
                        xt[:].rearrange("p (k t) -> p k t", t=512),
                        xT[:, c0:c1].rearrange("(k p) t -> p k t", p=128))
                return xt

            xt0 = xt_dma(0, split=True)
            weight_dmas_front()
            kvr0 = proj_part(0, xt0)
            xt_next = xt_dma(1)
            rts0 = rope_core(0, kvr0)
            norm_finish(0, rts0)
            weight_dmas_back()
            for n in range(NQ):
                attention_head(n, 0)
                if n + 1 < NQ:
                    kvr_next = proj_part(n + 1, xt_next)
                attention_head(n, 1)
                if n + 1 < NQ:
                    rts_next = rope_core(n + 1, kvr_next)
                attention_head(n, 2)
                if n + 2 < NQ:
                    xt_next = xt_dma(n + 2)
                attention_head(n, 3)
                if n + 1 < NQ:
                    norm_finish(n + 1, rts_next)
                outproj(n)

    nc.compile()
    return nc


def _get_nc():
    if "nc" not in _CACHE:
        _CACHE["nc"] = _build()
    return _CACHE["nc"]


def _make_masks():
    p = np.arange(128)[:, None]
    c = np.arange(128)[None, :]
    return (c >= p).astype(np.float32)


def _bf16(a):
    import ml_dtypes
    return np.ascontiguousarray(np.asarray(a).astype(ml_dtypes.bfloat16))


def kernel(x, cos, sin, Wq, Wk, Wv, Wo, _trace=False):
    from concourse.bass_utils import run_bass_kernel_spmd

    nc = _get_nc()
    c_ = np.asarray(cos)[0, :, 0, :].T.astype(np.float32)
    s_ = np.asarray(sin)[0, :, 0, :].T.astype(np.float32)
    cosT = _bf16(np.concatenate([c_, c_, c_, c_], axis=0))
    sinT = _bf16(np.concatenate([s_, s_, s_, s_], axis=0))
    # head-pair permutation of q columns within each 128-wide m-tile:
    # [hA.x1 | hB.x1 | hA.x2 | hB.x2]
    perm = np.zeros(256, dtype=np.int64)
    for mm in range(2):
        base = 128 * mm
        hA, hB = 128 * mm, 128 * mm + 64
        perm[base:base + 32] = hA + np.arange(32)
        perm[base + 32:base + 64] = hB + np.arange(32)
        perm[base + 64:base + 96] = hA + 32 + np.arange(32)
        perm[base + 96:base + 128] = hB + 32 + np.arange(32)
    masks = _bf16(_make_masks())
    eye = _bf16(np.eye(64, dtype=np.float32))
    in_maps = []
    for b in range(DP):
        xT = _bf16(np.asarray(x)[b].T)
        for s in range(TP):
            wkv = np.concatenate([np.asarray(Wk)[64 * s:64 * (s + 1), :],
                                  np.asarray(Wv)[64 * s:64 * (s + 1), :]],
                                 axis=0)
            in_maps.append({
                "xT": xT,
                "cosT": cosT,
                "sinT": sinT,
                "wqT": _bf16(np.asarray(Wq)[256 * s:256 * (s + 1), :]
                             .T[:, perm]),
                "wkvT": _bf16(wkv.T),
                "woT": _bf16(np.asarray(Wo)[:, 256 * s:256 * (s + 1)].T),
                "masks": masks,
                "eye64": eye,
            })
    res = run_bass_kernel_spmd(nc, in_maps, core_ids=list(range(8)),
                               trace=_trace)
    out = np.stack([
        np.concatenate([res.results[c]["outT"] for c in range(4)], axis=0).T,
        np.concatenate([res.results[c]["outT"] for c in range(4, 8)],
                       axis=0).T])
    if _trace:
        _CACHE["last_result"] = res
    return np.ascontiguousarray(out, dtype=np.float32)
